# revision 7
# baseline (speedup 1.0000x reference)
"""Trainium2 Bass kernel for nn_MultiHeadAttention_1812476199709.

Reference computation (per batch n of 8, H=8 heads, S=2048, head d=64):
    v = values.reshape(S,H,64) @ Wv.T ; k = keys... @ Wk.T ; q = query... @ Wq.T
    energy[h,q,k] = q.k ; masked where mask[k]==0 -> -1e20
    att = softmax(energy / sqrt(512)) ; out = (att @ v).reshape(S,512) @ Wo.T + bo
    returns (out, att)

Strategy: one batch element per NeuronCore (8 cores).  Host folds the Q/K
projections into a single 64x64 matrix A = Wq^T Wk / sqrt(512) (energy =
q_raw A k_raw^T) and folds Wv+Wo into per-head B_h = Wv^T Wo_h^T, so the
device only computes: scores^T (k-major) = (k A^T) q^T + mask-row via an
extra contraction row, exp on ACT into f32r, z^T = v^T att_u^T with a free
ones-column giving row sums s, PE-transposes of att_u back to q-major with a
fused x(1/s) normalize into the attention output, and a final projection
seeded with the bias via a K=1 matmul.
"""
import os
import threading

import numpy as np

import concourse.bass as bass
import concourse.tile as tile
from concourse import mybir
from concourse.bass_utils import run_bass_kernel_spmd
from concourse.masks import make_identity
from contextlib import ExitStack

F32 = mybir.dt.float32
F32R = mybir.dt.float32r
AF = mybir.ActivationFunctionType

H = 8          # heads
S = 2048       # sequence length
D = 64         # head size
E = 512        # embed dim
NKC = S // 128  # 16 k chunks
NQT = S // 128  # 16 q tiles
NQB = S // 512  # 4 q blocks
NEG = np.float32(-1e20 / np.sqrt(np.float32(E)))

_CTR = [0]


def _fix_multiwait(nc):
    """walrus here caps sync waits at 1/instruction (2 for EventSemaphore);
    Tile's tail drain can carry more.  Hoist excess waits onto standalone
    EventSemaphore instructions inserted before, same engine."""
    for fn in nc.m.functions:
        for bb in fn.blocks:
            insts = list(bb.instructions)
            out = []
            changed = False
            for ins in insts:
                si = ins.sync_info
                waits = list(si.on_wait) if si is not None else []
                cap = 2 if isinstance(ins, mybir.InstEventSemaphore) else 1
                if len(waits) > cap:
                    extra = waits[: len(waits) - cap]
                    keep = waits[len(waits) - cap:]
                    for i in range(0, len(extra), 2):
                        _CTR[0] += 1
                        ev = mybir.InstEventSemaphore(
                            name=f"W-fixwait-{_CTR[0]}", ins=[], outs=[],
                            sync_info=mybir.SyncInfo(
                                on_wait=list(extra[i:i + 2]), on_update=[]))
                        ev.engine = ins.engine
                        out.append(ev)
                    ins.sync_info = mybir.SyncInfo(
                        on_wait=keep, on_update=list(si.on_update))
                    changed = True
                out.append(ins)
            if changed:
                bb.instructions = out
    return nc


def build_nc():
    nc = bass.Bass("TRN2", target_bir_lowering=False, debug=False,
                   enable_asserts=False)
    q_d = nc.dram_tensor("q", [S, E], F32, kind="ExternalInput").ap()
    k_d = nc.dram_tensor("k", [S, E], F32, kind="ExternalInput").ap()
    v_d = nc.dram_tensor("v", [S, E], F32, kind="ExternalInput").ap()
    madd_d = nc.dram_tensor("madd", [1, S], F32, kind="ExternalInput").ap()
    apt_d = nc.dram_tensor("Apt", [D, D], F32, kind="ExternalInput").ap()
    b_d = nc.dram_tensor("B", [D, H * E], F32, kind="ExternalInput").ap()
    bo_d = nc.dram_tensor("bo", [1, E], F32, kind="ExternalInput").ap()
    att_d = nc.dram_tensor("att", [H, S, S], F32, kind="ExternalOutput").ap()
    out_d = nc.dram_tensor("out", [S, E], F32, kind="ExternalOutput").ap()

    with tile.TileContext(nc) as tc, ExitStack() as top:
        consts = top.enter_context(tc.tile_pool(name="consts", bufs=1))
        vpool = top.enter_context(tc.tile_pool(name="vpool", bufs=16))
        dram = top.enter_context(tc.tile_pool(name="dram", bufs=1, space="DRAM"))

        # ---- constants ----
        ident_f = consts.tile([128, 128], F32)
        make_identity(nc, ident_f[:])
        ident_r = consts.tile([128, 128], F32R)
        nc.vector.tensor_copy(ident_r[:], ident_f[:])
        ones_row = consts.tile([1, S], F32)
        nc.vector.memset(ones_row[:], 1.0)
        ones_c8 = consts.tile([128, 8], F32)
        nc.vector.memset(ones_c8[:], 1.0)
        one_f1 = consts.tile([1, 1], F32)
        nc.vector.memset(one_f1[:], 1.0)
        one_r = consts.tile([1, 128], F32R)
        nc.vector.tensor_copy(one_r[:], ones_row[0:1, 0:128])

        apt_r = consts.tile([D, D], F32R)
        madd_f = consts.tile([1, S], F32)
        nc.sync.dma_start(madd_f[:], madd_d)

        # ---- v tiles: (128, h, 65) per k-chunk, col 64 = ones (row-sum trick)
        v_tiles = []
        with tc.tile_pool(name="vstage", bufs=3) as vstage:
            apt_f = vstage.tile([D, D], F32, tag="apt_f")
            nc.sync.dma_start(apt_f[:], apt_d)
            nc.vector.tensor_copy(apt_r[:], apt_f[:])
            for kc in range(NKC):
                v_st = vstage.tile([128, E], F32)
                nc.sync.dma_start(v_st[:], v_d[kc * 128:(kc + 1) * 128, :])
                v_sb = vpool.tile([128, H, D + 1], F32R)
                nc.vector.tensor_copy(
                    v_sb[:, :, 0:D],
                    v_st[:].rearrange("p (h d) -> p h d", h=H))
                nc.vector.tensor_copy(v_sb[:, :, D:D + 1],
                                      ones_c8[:].rearrange("p (h o) -> p h o", o=1))
                v_tiles.append(v_sb)

        zt_dram = dram.tile([H, D, S], F32R)

        with (
            tc.tile_pool(name="stage", bufs=4) as stage,
            tc.tile_pool(name="qTp", bufs=1) as qTpool,
            tc.tile_pool(name="kATp", bufs=1) as kATpool,
            tc.tile_pool(name="kTst", bufs=1) as kTstage,
            tc.tile_pool(name="attp", bufs=1) as attpool,
            tc.tile_pool(name="attout", bufs=2) as outpool,
            tc.tile_pool(name="zsp", bufs=2) as zspool,
            tc.tile_pool(name="small", bufs=2) as small,
            tc.tile_pool(name="ztsb", bufs=2) as ztsb,
            tc.tile_pool(name="ps_sc", bufs=2, space="PSUM") as ps_sc,
            tc.tile_pool(name="ps_at", bufs=2, space="PSUM") as ps_at,
            tc.tile_pool(name="ps_zm", bufs=2, space="PSUM") as ps_zm,
        ):
            for hp in range(H // 2):  # head pairs
                # -- transpose raw q, k head-pair columns to (64, S) layout
                qTp = [qTpool.tile([D + 1, S], F32R, name=f"qTp{j}", tag=f"qTp{j}") for j in range(2)]
                kTs = [kTstage.tile([D, S], F32R, name=f"kTs{j}", tag=f"kTs{j}") for j in range(2)]
                for name, x_d, dsts in (("q", q_d, qTp), ("k", k_d, kTs)):
                    for sc in range(S // 128):
                        st = stage.tile([128, 128], F32)
                        nc.sync.dma_start(
                            st[:], x_d[sc * 128:(sc + 1) * 128,
                                       hp * 128:(hp + 1) * 128])
                        tr = ps_at.tile([128, 128], F32, tag="at")
                        nc.tensor.transpose(tr[:], st[:], ident_f[:])
                        for j in range(2):
                            nc.vector.tensor_copy(
                                dsts[j][0:D, sc * 128:(sc + 1) * 128],
                                tr[j * D:(j + 1) * D, :])
                for j in range(2):
                    nc.vector.tensor_copy(qTp[j][D:D + 1, :], ones_row[:])

                # -- kAT = A^T-projected k, plus mask row
                kATp = [kATpool.tile([D + 1, S], F32R, name=f"kATp{j}", tag=f"kATp{j}") for j in range(2)]
                for j in range(2):
                    for cb in range(S // 512):
                        ka = ps_at.tile([D, 512], F32, tag="at")
                        nc.tensor.matmul(
                            ka[:], apt_r[:],
                            kTs[j][:, cb * 512:(cb + 1) * 512],
                            start=True, stop=True)
                        nc.vector.tensor_copy(
                            kATp[j][0:D, cb * 512:(cb + 1) * 512], ka[:])
                    nc.vector.tensor_copy(kATp[j][D:D + 1, :], madd_f[:])

                for j in range(2):
                    h = 2 * hp + j
                    for qb in range(NQB):
                        # scores^T + exp -> attTu (128k x [16 chunks] x 512q)
                        attTu = attpool.tile([128, NKC, 512], F32R)
                        for g in range(NKC // 2):
                            sc_ps = ps_sc.tile([128, 1024], F32)
                            for u in range(2):
                                kc = 2 * g + u
                                nc.tensor.matmul(
                                    sc_ps[:, u * 512:(u + 1) * 512],
                                    kATp[j][:, kc * 128:(kc + 1) * 128],
                                    qTp[j][:, qb * 512:(qb + 1) * 512],
                                    start=True, stop=True)
                            nc.scalar.activation(
                                out=attTu[:, 2 * g:2 * g + 2, :],
                                in_=sc_ps[:], func=AF.Exp)
                        # z^T (+ row sums s in row 64)
                        zs_ps = ps_zm.tile([D + 1, 512], F32, tag="zm")
                        for kc in range(NKC):
                            nc.tensor.matmul(
                                zs_ps[:], v_tiles[kc][:, h, :],
                                attTu[:, kc, :],
                                start=(kc == 0), stop=(kc == NKC - 1))
                        zs_sb = zspool.tile([D + 1, 512], F32)
                        nc.vector.tensor_copy(zs_sb[:], zs_ps[:])
                        # rs[q] = 1/s[q] as per-partition column (128, 4)
                        s_row = small.tile([1, 512], F32, tag="s_row")
                        nc.vector.tensor_copy(s_row[:], zs_sb[D:D + 1, :])
                        rs_ps = ps_zm.tile([128, 4], F32, tag="zm")
                        for qt in range(4):
                            nc.tensor.matmul(
                                rs_ps[:, qt:qt + 1],
                                s_row[0:1, qt * 128:(qt + 1) * 128],
                                one_f1[:], start=True, stop=True)
                        rs_sb = small.tile([128, 4], F32)
                        nc.vector.reciprocal(rs_sb[:], rs_ps[:])

                        # transpose att_u to q-major + fused normalize
                        for qt in range(4):
                            att_out = outpool.tile([128, S], F32)
                            for kb in range(4):
                                at_ps = ps_at.tile([128, 512], F32R, tag="at")
                                for u in range(4):
                                    kc = kb * 4 + u
                                    nc.tensor.transpose(
                                        at_ps[:, u * 128:(u + 1) * 128],
                                        attTu[:, kc, qt * 128:(qt + 1) * 128],
                                        ident_r[:])
                                dst = att_out[:, kb * 512:(kb + 1) * 512]
                                src = at_ps[:].bitcast(F32)
                                if qt % 2 == 0:
                                    nc.vector.tensor_scalar_mul(
                                        dst, src, rs_sb[:, qt:qt + 1])
                                else:
                                    nc.scalar.mul(dst, src,
                                                  rs_sb[:, qt:qt + 1])
                            nc.sync.dma_start(
                                att_d[h, qb * 512 + qt * 128:
                                      qb * 512 + (qt + 1) * 128, :],
                                att_out[:])

                        # z fixup: normalize + transpose to (64, q) f32r
                        zts = ztsb.tile([D, 512], F32R)
                        for qt in range(4):
                            zq_ps = ps_zm.tile([128, D + 1], F32, tag="zm")
                            nc.tensor.transpose(
                                zq_ps[:], zs_sb[:, qt * 128:(qt + 1) * 128],
                                ident_f[0:D + 1, 0:D + 1])
                            z_n = small.tile([128, D], F32)
                            nc.vector.tensor_scalar_mul(
                                z_n[:], zq_ps[:, 0:D], rs_sb[:, qt:qt + 1])
                            zt_ps = ps_zm.tile([D, 128], F32, tag="zm")
                            nc.tensor.transpose(zt_ps[:], z_n[:], ident_f[:])
                            nc.vector.tensor_copy(
                                zts[:, qt * 128:(qt + 1) * 128], zt_ps[:])
                        nc.sync.dma_start(
                            zt_dram[h, :, qb * 512:(qb + 1) * 512], zts[:])

        # ---- final projection: out = bo + sum_h zT_h^T B_h ----
        with (
            tc.tile_pool(name="fsb", bufs=16) as fsb,
            tc.tile_pool(name="fconst", bufs=1) as fconst,
            tc.tile_pool(name="fout", bufs=3) as fout,
            tc.tile_pool(name="ps_f", bufs=2, space="PSUM") as ps_f,
        ):
            b_f = fconst.tile([D, H * E], F32)
            nc.sync.dma_start(b_f[:], b_d)
            b_r = fconst.tile([D, H * E], F32R)
            nc.vector.tensor_copy(b_r[:], b_f[:])
            bo_f = fconst.tile([1, E], F32)
            nc.sync.dma_start(bo_f[:], bo_d)
            bo_r = fconst.tile([1, E], F32R)
            nc.vector.tensor_copy(bo_r[:], bo_f[:])
            for qt in range(NQT):
                out_ps = ps_f.tile([128, E], F32)
                nc.tensor.matmul(out_ps[:], one_r[:], bo_r[:],
                                 start=True, stop=False)
                for h in range(H):
                    zt_t = fsb.tile([D, 128], F32R)
                    nc.sync.dma_start(
                        zt_t[:], zt_dram[h, :, qt * 128:(qt + 1) * 128])
                    nc.tensor.matmul(out_ps[:], zt_t[:],
                                     b_r[:, h * E:(h + 1) * E],
                                     start=False, stop=(h == H - 1))
                out_sb = fout.tile([128, E], F32)
                nc.vector.tensor_copy(out_sb[:], out_ps[:])
                nc.sync.dma_start(out_d[qt * 128:(qt + 1) * 128, :],
                                  out_sb[:])

    _fix_multiwait(nc)
    return nc


_cache = threading.local()


def _get_nc():
    nc = getattr(_cache, "nc", None)
    if nc is None:
        nc = build_nc()
        _cache.nc = nc
    return nc


def _prep_inputs(values, keys, query, mask, Wv, Wk, Wq, Wo, bo):
    T = np.sqrt(np.float32(E))
    A = (Wq.T @ Wk).astype(np.float32) / T          # (64, 64)
    Apt = np.ascontiguousarray(A.T)                  # lhsT for kAT matmul
    # B_h = Wv^T @ Wo_h^T ; concat along columns -> (64, H*512)
    B = np.concatenate(
        [Wv.T @ Wo[:, h * D:(h + 1) * D].T for h in range(H)], axis=1
    ).astype(np.float32)
    bo_row = np.ascontiguousarray(bo.reshape(1, E)).astype(np.float32)
    madd = np.where(mask == 0, NEG, np.float32(0.0)).astype(np.float32)
    in_maps = []
    for i in range(mask.shape[0]):
        in_maps.append({
            "q": np.ascontiguousarray(query[i]),
            "k": np.ascontiguousarray(keys[i]),
            "v": np.ascontiguousarray(values[i]),
            "madd": np.ascontiguousarray(madd[i].reshape(1, S)),
            "Apt": Apt, "B": B, "bo": bo_row,
        })
    return in_maps


def kernel(values, keys, query, mask, Wv, Wk, Wq, Wo, bo, _trace=False,
           _trace_kwargs=None):
    values = np.asarray(values, dtype=np.float32)
    keys = np.asarray(keys, dtype=np.float32)
    query = np.asarray(query, dtype=np.float32)
    mask = np.asarray(mask)
    Wv = np.asarray(Wv, dtype=np.float32)
    Wk = np.asarray(Wk, dtype=np.float32)
    Wq = np.asarray(Wq, dtype=np.float32)
    Wo = np.asarray(Wo, dtype=np.float32)
    bo = np.asarray(bo, dtype=np.float32)

    n = values.shape[0]
    nc = _get_nc()
    in_maps = _prep_inputs(values, keys, query, mask, Wv, Wk, Wq, Wo, bo)
    kwargs = dict(_trace_kwargs or {})
    res = run_bass_kernel_spmd(nc, in_maps, core_ids=list(range(n)),
                               trace=_trace, **kwargs)
    out = np.stack([r["out"] for r in res.results])
    att = np.stack([r["att"] for r in res.results])
    kernel.last_results = res
    return out, att
